# revision 9
# baseline (speedup 1.0000x reference)
"""Dense all-expert MoE (SwiGLU) kernel for Trainium2, expert-parallel over 8 cores.

Computes: out = sum_e silu(x @ Wg[e]) * (x @ Wu[e]) @ Wd[e]
with x: [B=2, S=2048, H=1024], Wg/Wu: [8, 1024, 4096], Wd: [8, 4096, 1024].

Sharding: expert-parallel. Core e gets expert e's weights plus the full token
set; each core produces a partial [T, H] output which the host sums.

Per-core kernel (bf16 matmul inputs, fp32 PSUM accumulation):
  stage A: hT[f, :, tokens] = silu(Wg_f^T @ xT) * (Wu_f^T @ xT)   (F on partitions)
  stage B: out[tokens, h]  += hT[f]^T @ Wd_f                      (tokens on partitions)
Host pre-lays-out all operands so every DMA lands 16KB-contiguous per
partition (HWDGE is descriptor-rate bound: 2KB packets cap a ring at
~130-180 GB/s, which starves the cold start):
  xQ  [NT, 128, KB, TB]        xQ[t, p, k, tt]       = x[t*TB+tt, 128k+p]
  wQ  [8, 128, 2, FQ, 1024]    wQ[i, p, 0, f2, k*128+m] = Wg[128k+p, 128(4i+f2)+m]
                               wQ[i, p, 1, f2, ...]  = same for Wu
  wdQ [2, 128, FB*512]         wdQ[h2, p, f*512+h]   = Wd[128f+p, 512*h2+h]

Perf notes (from NTFF traces of the previous version, 1358.9us):
  - PE sem-wait bubbles (~57ns each) charge per weight-TILE switch, so wg+wu
    for FOUR f-slices ship in ONE tile: 8 waits/block instead of 32.
  - The gpsimd SWDGE ring costs a ~4.3us DRAIN at kernel end; all DMA now
    rides the sync/scalar/vector HWDGE rings only.
  - Stage B runs m-outer/f-inner so each accumulator bank finishes and is
    evicted while the next accumulates; the kernel tail after the last MM is
    one [128,512] copy + one 128KB DMA instead of a bunched 4x drain.
  - The SwiGLU mul is split into 512-col halves with the u-matmuls c-outer,
    so stage B's first LDWEIGHTS (which Tile gates on the DVE clock) is
    satisfied before stage A's last matmul retires.
  - wd preload is spread over stage-A weight tiles 2..7 of block 0 so it
    never delays the first wg/wu (and the first x chunks) at cold start.
  - Output is bf16 (host sums partials in fp32): halves the eviction DMA.
"""

import numpy as np
import ml_dtypes

T = 4096          # B*S tokens
H = 1024          # hidden
F = 4096          # ffn
E = 8             # experts
N_CORES = 8
TB = 1024         # tokens per block
NT = T // TB      # 4 token blocks
KB = H // 128     # 8 hidden slices
FB = F // 128     # 32 ffn slices
FQ = 4            # f-slices per stage-A weight tile

_CACHE = {}


def _build_module():
    from contextlib import ExitStack

    import concourse.bass as bass
    import concourse.mybir as mybir
    import concourse.tile as tile
    from concourse import bacc

    f32 = mybir.dt.float32
    bf16 = mybir.dt.bfloat16

    nc = bacc.Bacc(
        "TRN2",
        target_bir_lowering=False,
        debug=False,
        enable_asserts=False,
        num_devices=N_CORES,
    )

    xQ = nc.dram_tensor("xQ", [NT, 128, KB, TB], bf16, kind="ExternalInput").ap()
    wQ = nc.dram_tensor(
        "wQ", [FB // FQ, 128, 2, FQ, KB * 128], bf16, kind="ExternalInput"
    ).ap()
    wdQ = nc.dram_tensor("wdQ", [2, 128, FB * 512], bf16, kind="ExternalInput").ap()
    out = nc.dram_tensor("out", [T, H], bf16, kind="ExternalOutput").ap()

    # wd preload schedule: weight-tile index fq -> (wdp half, col range) of a
    # 2MB 16KB-packet DMA; all four land well before stage B reads each half
    wd_sched = {
        8: (0, 0), 12: (0, 1), 16: (1, 0), 20: (1, 1),
    }

    with tile.TileContext(nc) as tc, ExitStack() as ctx:
        xpool = ctx.enter_context(tc.tile_pool(name="xpool", bufs=1))
        wpool = ctx.enter_context(tc.tile_pool(name="wpool", bufs=2))
        dpool = ctx.enter_context(tc.tile_pool(name="dpool", bufs=1))
        hpool = ctx.enter_context(tc.tile_pool(name="hpool", bufs=1))
        spool = ctx.enter_context(tc.tile_pool(name="spool", bufs=2))
        opool = ctx.enter_context(tc.tile_pool(name="opool", bufs=4))
        cpool = ctx.enter_context(tc.tile_pool(name="cpool", bufs=1))
        # one psum pool, 4 tags x [128,1024] (2 banks each) = all 8 banks;
        # stage A uses p0/p1 (even f) and p2/p3 (odd f) as g/u,
        # stage B uses p0..p3 as the 4 double-bank accumulators
        psum = ctx.enter_context(tc.tile_pool(name="psum", bufs=1, space="PSUM"))

        bias0 = cpool.tile([128, 1], f32, tag="bias0")
        nc.vector.memset(bias0[:], 0.0)

        # Wd stays resident in SBUF for the whole kernel (2 x 32KB/partition)
        wdp = [
            dpool.tile([128, FB * 512], bf16, tag=f"wdp{h2}", name=f"wdp{h2}")
            for h2 in range(H // 512)
        ]

        xbs = {}
        for t in range(NT):
            # ---- stage A: hT[f] = silu(Wg_f^T xT) * (Wu_f^T xT), F on partitions
            if t == 0:
                xb = xpool.tile([128, KB, TB], bf16, tag="xb")
                # cold start: the block rides the (otherwise idle) scalar
                # HWDGE ring; k=0/1 land alone so the first matmuls start
                # ASAP while weights stream on the sync ring
                nc.scalar.dma_start(xb[:, 0, :], xQ[0][:, 0, :])
                nc.scalar.dma_start(xb[:, 1, :], xQ[0][:, 1, :])
                nc.scalar.dma_start(xb[:, 2:KB, :], xQ[0][:, 2:KB, :])
            else:
                xb = xbs.pop(t)

            hts = []
            for fq in range(0, FB, FQ):
                # combined wg+wu tile for FQ f-slices: ONE PE sem-wait per
                # FQ slices, 16KB-contiguous per partition
                wt = wpool.tile([128, 2, FQ, KB * 128], bf16, tag="w")
                qi = fq // FQ
                if t == 0 and fq == 0:
                    # cold start: land (g,f=0) then (u,f=0) first so the
                    # first accumulation group only waits for 256KB
                    nc.sync.dma_start(wt[:, 0, 0, :], wQ[0][:, 0, 0, :])
                    nc.sync.dma_start(wt[:, 1, 0, :], wQ[0][:, 1, 0, :])
                    nc.sync.dma_start(wt[:, 0, 1:FQ, :], wQ[0][:, 0, 1:FQ, :])
                    nc.sync.dma_start(wt[:, 1, 1:FQ, :], wQ[0][:, 1, 1:FQ, :])
                else:
                    nc.sync.dma_start(wt[:], wQ[qi])
                if t == 0 and fq in wd_sched:
                    # wd preload spread over mid-block tiles: off the critical
                    # cold-start path, done before stage B needs each half
                    h2i, ci = wd_sched[fq]
                    sl = slice(ci * FB * 256, (ci + 1) * FB * 256)
                    nc.sync.dma_start(wdp[h2i][:, sl], wdQ[h2i][:, sl])

                for f2 in range(FQ):
                    f = fq + f2
                    g = psum.tile([128, TB], f32, tag=f"p{(f % 2) * 2}")
                    for k in range(KB):
                        for c in range(TB // 512):
                            nc.tensor.matmul(
                                g[:, c * 512 : (c + 1) * 512],
                                wt[:, 0, f2, k * 128 : (k + 1) * 128],
                                xb[:, k, c * 512 : (c + 1) * 512],
                                start=(k == 0),
                                stop=(k == KB - 1),
                            )
                    sil = spool.tile([128, TB], f32, tag="sil")
                    nc.scalar.activation(
                        sil[:], g[:], mybir.ActivationFunctionType.Silu, bias=bias0[:]
                    )

                    # u runs c-outer so each 512-col half of the product is
                    # ready as soon as its 8 k-accumulation matmuls retire
                    u = psum.tile([128, TB], f32, tag=f"p{(f % 2) * 2 + 1}")
                    ht = hpool.tile([128, TB], bf16, tag=f"h{f}")
                    for c in range(TB // 512):
                        sl = slice(c * 512, (c + 1) * 512)
                        for k in range(KB):
                            nc.tensor.matmul(
                                u[:, sl],
                                wt[:, 1, f2, k * 128 : (k + 1) * 128],
                                xb[:, k, sl],
                                start=(k == 0),
                                stop=(k == KB - 1),
                            )
                        nc.vector.tensor_mul(ht[:, sl], sil[:, sl], u[:, sl])
                    hts.append(ht)

            if t + 1 < NT:
                # prefetch next block's x now: the trigger lands on the sync
                # ring AHEAD of this block's output-DMA triggers (whose deps
                # only resolve late in stage B), so the 2MB transfer runs at
                # the start of stage B instead of just-in-time at its end
                nxb = xpool.tile([128, KB, TB], bf16, tag="xb")
                nc.sync.dma_start(nxb[:], xQ[t + 1])
                xbs[t + 1] = nxb

            # ---- stage B: out[tokens, h] += hT^T @ Wd, tokens on partitions
            # m-outer/f-inner: each 512-col accumulator half finishes its full
            # f-contraction and is evicted while the next half accumulates, so
            # psum banks free progressively (next stage A / kernel tail never
            # wait on a bunched drain)
            for h2 in range(H // 512):
                for i in range(4):
                    acc = psum.tile([128, TB], f32, tag=f"p{i}", name=f"acc{t}_{h2}_{i}")
                    for half in range(2):
                        m = 2 * i + half
                        asl = slice(half * 512, half * 512 + 512)
                        for f in range(FB):
                            nc.tensor.matmul(
                                acc[:, asl],
                                hts[f][:, m * 128 : (m + 1) * 128],
                                wdp[h2][:, f * 512 : (f + 1) * 512],
                                start=(f == 0),
                                stop=(f == FB - 1),
                            )
                        ob = opool.tile([128, 512], bf16, tag="ob")
                        row = t * TB + m * 128
                        dst = out[row : row + 128, h2 * 512 : (h2 + 1) * 512]
                        if half == 0:
                            nc.vector.tensor_copy(ob[:], acc[:, asl])
                            nc.sync.dma_start(dst, ob[:])
                        else:
                            nc.scalar.activation(
                                ob[:], acc[:, asl], mybir.ActivationFunctionType.Copy
                            )
                            nc.scalar.dma_start(dst, ob[:])

    nc.compile()
    return nc


def _get_module():
    if "nc" not in _CACHE:
        _CACHE["nc"] = _build_module()
    return _CACHE["nc"]


def _prep_inputs(hidden_states, Wg, Wu, Wd):
    bf16 = ml_dtypes.bfloat16
    x = np.asarray(hidden_states, dtype=np.float32).reshape(T, H)
    # xQ[t, p, k, tt] = x[t*TB+tt, 128k+p]
    xQ = np.ascontiguousarray(
        x.reshape(NT, TB, KB, 128).transpose(0, 3, 2, 1)
    ).astype(bf16)
    in_maps = []
    for e in range(N_CORES):
        # w[f, p, (k m)] = W[e, 128k+p, 128f+m], f-major tiles of FQ slices
        def _wslices(W):
            return (
                np.asarray(W, dtype=np.float32)
                .reshape(KB, 128, FB, 128)
                .transpose(2, 1, 0, 3)
                .reshape(FB // FQ, FQ, 128, KB * 128)
                .transpose(0, 2, 1, 3)  # [8, 128, FQ, 1024]
            )
        wQ = np.ascontiguousarray(
            np.stack([_wslices(Wg[e]), _wslices(Wu[e])], axis=2)
        ).astype(bf16)  # [8, 128, 2, FQ, 1024]
        # wdQ[h2, p, f*512+h] = Wd[e, 128f+p, 512*h2+h]
        wdQ = np.ascontiguousarray(
            np.asarray(Wd[e], dtype=np.float32)
            .reshape(FB, 128, 2, 512)
            .transpose(2, 1, 0, 3)
            .reshape(2, 128, FB * 512)
        ).astype(bf16)
        in_maps.append({"xQ": xQ, "wQ": wQ, "wdQ": wdQ})
    return in_maps


def _run(in_maps, trace=False, **kwargs):
    from concourse import bass_utils

    nc = _get_module()
    return bass_utils.run_bass_kernel_spmd(
        nc, in_maps, core_ids=list(range(N_CORES)), trace=trace, **kwargs
    )


def kernel(hidden_states, Wg, Wu, Wd):
    import time

    in_maps = _prep_inputs(hidden_states, Wg, Wu, Wd)
    last_exc = None
    for attempt in range(3):
        try:
            res = _run(in_maps)
            break
        except Exception as exc:  # transient device-unrecoverable wedges
            last_exc = exc
            time.sleep(5 * (attempt + 1))
    else:
        raise last_exc
    partials = np.stack(
        [np.asarray(r["out"], dtype=np.float32) for r in res.results], axis=0
    )
    total = partials.sum(axis=0, dtype=np.float32)
    return total.reshape(2, 2048, H).astype(np.float32)


# revision 19
# speedup vs baseline: 1.0024x; 1.0024x over previous
"""Dense all-expert MoE (SwiGLU) kernel for Trainium2, expert-parallel over 8 cores.

Computes: out = sum_e silu(x @ Wg[e]) * (x @ Wu[e]) @ Wd[e]
with x: [B=2, S=2048, H=1024], Wg/Wu: [8, 1024, 4096], Wd: [8, 4096, 1024].

Sharding: expert-parallel. Core e gets expert e's weights plus the full token
set; each core produces a partial [T, H] output which the host sums.

Per-core kernel (bf16 matmul inputs, fp32 PSUM accumulation):
  stage A: hT[f, :, tokens] = silu(Wg_f^T @ xT) * (Wu_f^T @ xT)   (F on partitions)
  stage B: out[tokens, h]  += hT[f]^T @ Wd_f                      (tokens on partitions)
Host pre-lays-out all operands so every DMA lands 16KB-contiguous per
partition (HWDGE is descriptor-rate bound: 2KB packets cap a ring at
~130-180 GB/s, which starves the cold start):
  xQ  [NT, 128, KB, TB]        xQ[t, p, k, tt]       = x[t*TB+tt, 128k+p]
  wQ  [8, 128, 2, FQ, 1024]    wQ[i, p, 0, f2, k*128+m] = Wg[128k+p, 128(4i+f2)+m]
                               wQ[i, p, 1, f2, ...]  = same for Wu
  wdQ [2, 128, FB*512]         wdQ[h2, p, f*512+h]   = Wd[128f+p, 512*h2+h]

Perf notes (from NTFF traces of the previous version, 1358.9us):
  - PE sem-wait bubbles (~57ns each) charge per weight-TILE switch, so wg+wu
    for FOUR f-slices ship in ONE tile: 8 waits/block instead of 32.
  - The gpsimd SWDGE ring costs a ~4.3us DRAIN at kernel end; all DMA now
    rides the sync/scalar/vector HWDGE rings only.
  - Stage B runs m-outer/f-inner so each accumulator bank finishes and is
    evicted while the next accumulates; the kernel tail after the last MM is
    one [128,512] copy + one 128KB DMA instead of a bunched 4x drain.
  - The SwiGLU mul is split into 512-col halves with the u-matmuls c-outer,
    so stage B's first LDWEIGHTS (which Tile gates on the DVE clock) is
    satisfied before stage A's last matmul retires.
  - wd preload is spread over stage-A weight tiles 2..7 of block 0 so it
    never delays the first wg/wu (and the first x chunks) at cold start.
  - Output is bf16 (host sums partials in fp32): halves the eviction DMA.
"""

import numpy as np
import ml_dtypes

T = 4096          # B*S tokens
H = 1024          # hidden
F = 4096          # ffn
E = 8             # experts
N_CORES = 8
TB = 1024         # tokens per block
NT = T // TB      # 4 token blocks
KB = H // 128     # 8 hidden slices
FB = F // 128     # 32 ffn slices
FQ = 4            # f-slices per stage-A weight tile

_CACHE = {}


def _build_module():
    from contextlib import ExitStack

    import concourse.bass as bass
    import concourse.mybir as mybir
    import concourse.tile as tile
    from concourse import bacc

    f32 = mybir.dt.float32
    bf16 = mybir.dt.bfloat16

    nc = bacc.Bacc(
        "TRN2",
        target_bir_lowering=False,
        debug=False,
        enable_asserts=False,
        num_devices=N_CORES,
    )

    xQ = nc.dram_tensor("xQ", [NT, 128, KB, TB], bf16, kind="ExternalInput").ap()
    wQ = nc.dram_tensor(
        "wQ", [FB // FQ, 128, 2, FQ, KB * 128], bf16, kind="ExternalInput"
    ).ap()
    wdQ = nc.dram_tensor("wdQ", [2, 128, FB * 512], bf16, kind="ExternalInput").ap()
    out = nc.dram_tensor("out", [T, H], bf16, kind="ExternalOutput").ap()

    # wd preload schedule: weight-tile index fq -> (wdp half, col range) of a
    # 2MB 16KB-packet DMA; all four land well before stage B reads each half
    wd_sched = {
        8: (0, 0), 12: (0, 1), 16: (1, 0), 20: (1, 1),
    }

    with tile.TileContext(nc) as tc, ExitStack() as ctx:
        xpool = ctx.enter_context(tc.tile_pool(name="xpool", bufs=1))
        wpool = ctx.enter_context(tc.tile_pool(name="wpool", bufs=2))
        dpool = ctx.enter_context(tc.tile_pool(name="dpool", bufs=1))
        hpool = ctx.enter_context(tc.tile_pool(name="hpool", bufs=1))
        spool = ctx.enter_context(tc.tile_pool(name="spool", bufs=2))
        opool = ctx.enter_context(tc.tile_pool(name="opool", bufs=4))
        cpool = ctx.enter_context(tc.tile_pool(name="cpool", bufs=1))
        # one psum pool, 8 single-bank [128,512] tags: per-bank tiles keep
        # Tile's WAR tracking at bank granularity (a start=True matmul in one
        # bank never waits an eviction copy of its neighbour).
        # stage A even f: g->(b0,b1) u->(b2,b3); odd f: b4..b7 (c halves).
        # stage B: accumulator m -> b{m}.
        psum = ctx.enter_context(tc.tile_pool(name="psum", bufs=1, space="PSUM"))

        bias0 = cpool.tile([128, 1], f32, tag="bias0")
        nc.vector.memset(bias0[:], 0.0)

        # Wd stays resident in SBUF for the whole kernel (2 x 32KB/partition)
        wdp = [
            dpool.tile([128, FB * 512], bf16, tag=f"wdp{h2}", name=f"wdp{h2}")
            for h2 in range(H // 512)
        ]

        xbs = {}
        for t in range(NT):
            # ---- stage A: hT[f] = silu(Wg_f^T xT) * (Wu_f^T xT), F on partitions
            if t == 0:
                # cold start: both HWDGE rings ramp slowly (~150KB/us), so
                # the first block's x and weights are issued as a deadline-
                # ordered interleave across BOTH rings: xb k-chunks alternate
                # scalar/sync between the weight chunks, matching the cold
                # (427ns/MM, k-outer) PE consumption order.
                xb = xpool.tile([128, KB, TB], bf16, tag="xb")
                wt0 = wpool.tile([128, 2, FQ, KB * 128], bf16, tag="w")
                nc.scalar.dma_start(xb[:, 0, :], xQ[0][:, 0, :])
                nc.sync.dma_start(wt0[:, 0, 0, :], wQ[0][:, 0, 0, :])  # g f0
                nc.scalar.dma_start(xb[:, 1, :], xQ[0][:, 1, :])
                nc.sync.dma_start(xb[:, 2, :], xQ[0][:, 2, :])
                nc.scalar.dma_start(xb[:, 3, :], xQ[0][:, 3, :])
                nc.sync.dma_start(xb[:, 4, :], xQ[0][:, 4, :])
                nc.scalar.dma_start(xb[:, 5, :], xQ[0][:, 5, :])
                nc.sync.dma_start(xb[:, 6, :], xQ[0][:, 6, :])
                nc.scalar.dma_start(xb[:, 7, :], xQ[0][:, 7, :])
                nc.sync.dma_start(wt0[:, 1, 0, :], wQ[0][:, 1, 0, :])  # u f0
                nc.sync.dma_start(wt0[:, 0, 1, :], wQ[0][:, 0, 1, :])  # g f1
                nc.sync.dma_start(wt0[:, 1, 1, :], wQ[0][:, 1, 1, :])  # u f1
                nc.sync.dma_start(wt0[:, 0, 2:FQ, :], wQ[0][:, 0, 2:FQ, :])
                nc.sync.dma_start(wt0[:, 1, 2:FQ, :], wQ[0][:, 1, 2:FQ, :])
                wts = {0: wt0}
            else:
                xb = xbs.pop(t)

            hts = []
            for fq in range(0, FB, FQ):
                # combined wg+wu tile for FQ f-slices: ONE PE sem-wait per
                # FQ slices, 16KB-contiguous per partition
                qi = fq // FQ
                if t == 0 and fq == 0:
                    wt = wts.pop(0)  # cold-start ladder issued above
                else:
                    wt = wpool.tile([128, 2, FQ, KB * 128], bf16, tag="w")
                    nc.sync.dma_start(wt[:], wQ[qi])
                if t == 0 and fq in wd_sched:
                    # wd preload spread over mid-block tiles: off the critical
                    # cold-start path, done before stage B needs each half
                    h2i, ci = wd_sched[fq]
                    sl = slice(ci * FB * 256, (ci + 1) * FB * 256)
                    nc.sync.dma_start(wdp[h2i][:, sl], wdQ[h2i][:, sl])

                for f2 in range(FQ):
                    f = fq + f2
                    b = (f % 2) * 4  # psum bank group: even f -> b0..b3, odd -> b4..b7
                    sil = spool.tile([128, TB], f32, tag="sil")
                    # g runs k-outer/c-inner: each xb k-chunk is consumed for
                    # two matmuls before the next is needed (cold start feeds
                    # at half the rate of a c-outer loop)
                    gc = [
                        psum.tile(
                            [128, 512], f32, tag=f"b{b + c}", name=f"g{t}_{f}_{c}"
                        )
                        for c in range(TB // 512)
                    ]
                    for k in range(KB):
                        for c in range(TB // 512):
                            nc.tensor.matmul(
                                gc[c][:],
                                wt[:, 0, f2, k * 128 : (k + 1) * 128],
                                xb[:, k, c * 512 : (c + 1) * 512],
                                start=(k == 0),
                                stop=(k == KB - 1),
                            )
                    for c in range(TB // 512):
                        sl = slice(c * 512, (c + 1) * 512)
                        nc.scalar.activation(
                            sil[:, sl], gc[c][:], mybir.ActivationFunctionType.Silu,
                            bias=bias0[:],
                        )

                    # u runs c-outer so each 512-col half of the product is
                    # ready as soon as its 8 k-accumulation matmuls retire
                    ht = hpool.tile([128, TB], bf16, tag=f"h{f}")
                    for c in range(TB // 512):
                        sl = slice(c * 512, (c + 1) * 512)
                        u = psum.tile([128, 512], f32, tag=f"b{b + 2 + c}")
                        for k in range(KB):
                            nc.tensor.matmul(
                                u[:],
                                wt[:, 1, f2, k * 128 : (k + 1) * 128],
                                xb[:, k, sl],
                                start=(k == 0),
                                stop=(k == KB - 1),
                            )
                        nc.vector.tensor_mul(ht[:, sl], sil[:, sl], u[:])
                    hts.append(ht)

            if t + 1 < NT:
                # prefetch next block's x now: the trigger lands on the sync
                # ring AHEAD of this block's output-DMA triggers (whose deps
                # only resolve late in stage B), so the 2MB transfer runs at
                # the start of stage B instead of just-in-time at its end
                nxb = xpool.tile([128, KB, TB], bf16, tag="xb")
                nc.sync.dma_start(nxb[:], xQ[t + 1])
                xbs[t + 1] = nxb

            # ---- stage B: out[tokens, h] += hT^T @ Wd, tokens on partitions
            # m-outer/f-inner: each single-bank accumulator finishes its full
            # f-contraction and is evicted while the next accumulates, so
            # psum banks free progressively (next stage A / kernel tail never
            # wait on a bunched drain)
            for h2 in range(H // 512):
                for m in range(8):
                    acc = psum.tile(
                        [128, 512], f32, tag=f"b{m}", name=f"acc{t}_{h2}_{m}"
                    )
                    for f in range(FB):
                        nc.tensor.matmul(
                            acc[:],
                            hts[f][:, m * 128 : (m + 1) * 128],
                            wdp[h2][:, f * 512 : (f + 1) * 512],
                            start=(f == 0),
                            stop=(f == FB - 1),
                        )
                    ob = opool.tile([128, 512], bf16, tag="ob")
                    row = t * TB + m * 128
                    dst = out[row : row + 128, h2 * 512 : (h2 + 1) * 512]
                    # m<4 on ACT+scalar ring, m>=4 on DVE+sync ring: the very
                    # last eviction (m=7) takes the cheaper DVE+sync path
                    if m < 4:
                        nc.scalar.activation(
                            ob[:], acc[:], mybir.ActivationFunctionType.Copy
                        )
                        nc.scalar.dma_start(dst, ob[:])
                    else:
                        nc.vector.tensor_copy(ob[:], acc[:])
                        nc.sync.dma_start(dst, ob[:])

    nc.compile()
    return nc


def _get_module():
    if "nc" not in _CACHE:
        _CACHE["nc"] = _build_module()
    return _CACHE["nc"]


def _prep_inputs(hidden_states, Wg, Wu, Wd):
    bf16 = ml_dtypes.bfloat16
    x = np.asarray(hidden_states, dtype=np.float32).reshape(T, H)
    # xQ[t, p, k, tt] = x[t*TB+tt, 128k+p]
    xQ = np.ascontiguousarray(
        x.reshape(NT, TB, KB, 128).transpose(0, 3, 2, 1)
    ).astype(bf16)
    in_maps = []
    for e in range(N_CORES):
        # w[f, p, (k m)] = W[e, 128k+p, 128f+m], f-major tiles of FQ slices
        def _wslices(W):
            return (
                np.asarray(W, dtype=np.float32)
                .reshape(KB, 128, FB, 128)
                .transpose(2, 1, 0, 3)
                .reshape(FB // FQ, FQ, 128, KB * 128)
                .transpose(0, 2, 1, 3)  # [8, 128, FQ, 1024]
            )
        wQ = np.ascontiguousarray(
            np.stack([_wslices(Wg[e]), _wslices(Wu[e])], axis=2)
        ).astype(bf16)  # [8, 128, 2, FQ, 1024]
        # wdQ[h2, p, f*512+h] = Wd[e, 128f+p, 512*h2+h]
        wdQ = np.ascontiguousarray(
            np.asarray(Wd[e], dtype=np.float32)
            .reshape(FB, 128, 2, 512)
            .transpose(2, 1, 0, 3)
            .reshape(2, 128, FB * 512)
        ).astype(bf16)
        in_maps.append({"xQ": xQ, "wQ": wQ, "wdQ": wdQ})
    return in_maps


def _run(in_maps, trace=False, **kwargs):
    from concourse import bass_utils

    nc = _get_module()
    return bass_utils.run_bass_kernel_spmd(
        nc, in_maps, core_ids=list(range(N_CORES)), trace=trace, **kwargs
    )


def kernel(hidden_states, Wg, Wu, Wd):
    import time

    in_maps = _prep_inputs(hidden_states, Wg, Wu, Wd)
    last_exc = None
    for attempt in range(3):
        try:
            res = _run(in_maps)
            break
        except Exception as exc:  # transient device-unrecoverable wedges
            last_exc = exc
            time.sleep(5 * (attempt + 1))
    else:
        raise last_exc
    partials = np.stack(
        [np.asarray(r["out"], dtype=np.float32) for r in res.results], axis=0
    )
    total = partials.sum(axis=0, dtype=np.float32)
    return total.reshape(2, 2048, H).astype(np.float32)


# revision 21
# speedup vs baseline: 1.0024x; 1.0000x over previous
"""Dense all-expert MoE (SwiGLU) kernel for Trainium2, expert-parallel over 8 cores.

Computes: out = sum_e silu(x @ Wg[e]) * (x @ Wu[e]) @ Wd[e]
with x: [B=2, S=2048, H=1024], Wg/Wu: [8, 1024, 4096], Wd: [8, 4096, 1024].

Sharding: expert-parallel. Core e gets expert e's weights plus the full token
set; each core produces a partial [T, H] output which the host sums.

Per-core kernel (bf16 matmul inputs, fp32 PSUM accumulation):
  stage A: hT[f, :, tokens] = silu(Wg_f^T @ xT) * (Wu_f^T @ xT)   (F on partitions)
  stage B: out[tokens, h]  += hT[f]^T @ Wd_f                      (tokens on partitions)
Host pre-lays-out all operands so every DMA lands 16KB-contiguous per
partition (HWDGE is descriptor-rate bound: 2KB packets cap a ring at
~130-180 GB/s, which starves the cold start):
  xQ  [NT, 128, KB, TB]        xQ[t, p, k, tt]       = x[t*TB+tt, 128k+p]
  wQ  [8, 128, 2, FQ, 1024]    wQ[i, p, 0, f2, k*128+m] = Wg[128k+p, 128(4i+f2)+m]
                               wQ[i, p, 1, f2, ...]  = same for Wu
  wdQ [2, 128, FB*512]         wdQ[h2, p, f*512+h]   = Wd[128f+p, 512*h2+h]

Perf notes (from NTFF traces of the previous version, 1358.9us):
  - PE sem-wait bubbles (~57ns each) charge per weight-TILE switch, so wg+wu
    for FOUR f-slices ship in ONE tile: 8 waits/block instead of 32.
  - The gpsimd SWDGE ring costs a ~4.3us DRAIN at kernel end; all DMA now
    rides the sync/scalar/vector HWDGE rings only.
  - Stage B runs m-outer/f-inner so each accumulator bank finishes and is
    evicted while the next accumulates; the kernel tail after the last MM is
    one [128,512] copy + one 128KB DMA instead of a bunched 4x drain.
  - The SwiGLU mul is split into 512-col halves with the u-matmuls c-outer,
    so stage B's first LDWEIGHTS (which Tile gates on the DVE clock) is
    satisfied before stage A's last matmul retires.
  - wd preload is spread over stage-A weight tiles 2..7 of block 0 so it
    never delays the first wg/wu (and the first x chunks) at cold start.
  - Output is bf16 (host sums partials in fp32): halves the eviction DMA.
"""

import numpy as np
import ml_dtypes

T = 4096          # B*S tokens
H = 1024          # hidden
F = 4096          # ffn
E = 8             # experts
N_CORES = 8
TB = 1024         # tokens per block
NT = T // TB      # 4 token blocks
KB = H // 128     # 8 hidden slices
FB = F // 128     # 32 ffn slices
FQ = 4            # f-slices per stage-A weight tile

_CACHE = {}


def _build_module():
    from contextlib import ExitStack

    import concourse.bass as bass
    import concourse.mybir as mybir
    import concourse.tile as tile
    from concourse import bacc

    f32 = mybir.dt.float32
    bf16 = mybir.dt.bfloat16

    nc = bacc.Bacc(
        "TRN2",
        target_bir_lowering=False,
        debug=False,
        enable_asserts=False,
        num_devices=N_CORES,
    )

    xQ = nc.dram_tensor("xQ", [NT, 128, KB, TB], bf16, kind="ExternalInput").ap()
    wQ = nc.dram_tensor(
        "wQ", [FB // FQ, 128, 2, FQ, KB * 128], bf16, kind="ExternalInput"
    ).ap()
    wdQ = nc.dram_tensor("wdQ", [2, 128, FB * 512], bf16, kind="ExternalInput").ap()
    out = nc.dram_tensor("out", [T, H], bf16, kind="ExternalOutput").ap()

    # wd preload schedule: weight-tile index fq -> (wdp half, col range) of a
    # 2MB 16KB-packet DMA; all four land well before stage B reads each half
    wd_sched = {
        8: (0, 0), 12: (0, 1), 16: (1, 0), 20: (1, 1),
    }

    with tile.TileContext(nc) as tc, ExitStack() as ctx:
        xpool = ctx.enter_context(tc.tile_pool(name="xpool", bufs=1))
        wpool = ctx.enter_context(tc.tile_pool(name="wpool", bufs=2))
        dpool = ctx.enter_context(tc.tile_pool(name="dpool", bufs=1))
        hpool = ctx.enter_context(tc.tile_pool(name="hpool", bufs=1))
        spool = ctx.enter_context(tc.tile_pool(name="spool", bufs=2))
        opool = ctx.enter_context(tc.tile_pool(name="opool", bufs=4))
        cpool = ctx.enter_context(tc.tile_pool(name="cpool", bufs=1))
        # one psum pool, 8 single-bank [128,512] tags: per-bank tiles keep
        # Tile's WAR tracking at bank granularity (a start=True matmul in one
        # bank never waits an eviction copy of its neighbour).
        # stage A even f: g->(b0,b1) u->(b2,b3); odd f: b4..b7 (c halves).
        # stage B: accumulator m -> b{m}.
        psum = ctx.enter_context(tc.tile_pool(name="psum", bufs=1, space="PSUM"))

        bias0 = cpool.tile([128, 1], f32, tag="bias0")
        nc.vector.memset(bias0[:], 0.0)

        # HAM warm-up: the PE clock-gate releases only after ~3.4us of
        # sustained activity. The first real matmul can't start before the
        # prologue + DMA-ring ramp (~10-14us, run-variable), so burn that
        # window on dummy N=128 matmuls into a scratch bank — the first real
        # matmuls then issue at 2.4GHz instead of 1.2GHz.
        warm = cpool.tile([128, 256], bf16, tag="warm")
        nc.vector.memset(warm[:], 0.0)
        wps = psum.tile([128, 512], f32, tag="b7", name="warmps")
        for i in range(56):
            nc.tensor.matmul(
                wps[:, 0:128], warm[:, 0:128], warm[:, 128:256],
                start=True, stop=True,
            )

        # Wd stays resident in SBUF for the whole kernel (2 x 32KB/partition)
        wdp = [
            dpool.tile([128, FB * 512], bf16, tag=f"wdp{h2}", name=f"wdp{h2}")
            for h2 in range(H // 512)
        ]

        xbs = {}
        for t in range(NT):
            # ---- stage A: hT[f] = silu(Wg_f^T xT) * (Wu_f^T xT), F on partitions
            if t == 0:
                # cold start: both HWDGE rings ramp slowly (~150KB/us), so
                # the first block's x and weights are issued as a deadline-
                # ordered interleave across BOTH rings: xb k-chunks alternate
                # scalar/sync between the weight chunks, matching the cold
                # (427ns/MM, k-outer) PE consumption order.
                xb = xpool.tile([128, KB, TB], bf16, tag="xb")
                wt0 = wpool.tile([128, 2, FQ, KB * 128], bf16, tag="w")
                # x on the scalar ring with escalating chunk sizes (bigger
                # chunks = bigger packets = faster per byte on the ramping
                # ring); weights on the sync ring in consumption order
                nc.scalar.dma_start(xb[:, 0, :], xQ[0][:, 0, :])
                nc.sync.dma_start(wt0[:, 0, 0, :], wQ[0][:, 0, 0, :])  # g f0
                nc.scalar.dma_start(xb[:, 1, :], xQ[0][:, 1, :])
                nc.sync.dma_start(wt0[:, 1, 0, :], wQ[0][:, 1, 0, :])  # u f0
                nc.scalar.dma_start(xb[:, 2:4, :], xQ[0][:, 2:4, :])
                nc.sync.dma_start(wt0[:, 0, 1, :], wQ[0][:, 0, 1, :])  # g f1
                nc.scalar.dma_start(xb[:, 4:KB, :], xQ[0][:, 4:KB, :])
                nc.sync.dma_start(wt0[:, 1, 1, :], wQ[0][:, 1, 1, :])  # u f1
                nc.sync.dma_start(wt0[:, 0, 2:FQ, :], wQ[0][:, 0, 2:FQ, :])
                nc.sync.dma_start(wt0[:, 1, 2:FQ, :], wQ[0][:, 1, 2:FQ, :])
                wts = {0: wt0}
            else:
                xb = xbs.pop(t)

            hts = []
            for fq in range(0, FB, FQ):
                # combined wg+wu tile for FQ f-slices: ONE PE sem-wait per
                # FQ slices, 16KB-contiguous per partition
                qi = fq // FQ
                if t == 0 and fq == 0:
                    wt = wts.pop(0)  # cold-start ladder issued above
                else:
                    wt = wpool.tile([128, 2, FQ, KB * 128], bf16, tag="w")
                    nc.sync.dma_start(wt[:], wQ[qi])
                if t == 0 and fq in wd_sched:
                    # wd preload spread over mid-block tiles: off the critical
                    # cold-start path, done before stage B needs each half
                    h2i, ci = wd_sched[fq]
                    sl = slice(ci * FB * 256, (ci + 1) * FB * 256)
                    nc.sync.dma_start(wdp[h2i][:, sl], wdQ[h2i][:, sl])

                for f2 in range(FQ):
                    f = fq + f2
                    b = (f % 2) * 4  # psum bank group: even f -> b0..b3, odd -> b4..b7
                    sil = spool.tile([128, TB], f32, tag="sil")
                    # g runs k-outer/c-inner: each xb k-chunk is consumed for
                    # two matmuls before the next is needed (cold start feeds
                    # at half the rate of a c-outer loop)
                    gc = [
                        psum.tile(
                            [128, 512], f32, tag=f"b{b + c}", name=f"g{t}_{f}_{c}"
                        )
                        for c in range(TB // 512)
                    ]
                    for k in range(KB):
                        for c in range(TB // 512):
                            nc.tensor.matmul(
                                gc[c][:],
                                wt[:, 0, f2, k * 128 : (k + 1) * 128],
                                xb[:, k, c * 512 : (c + 1) * 512],
                                start=(k == 0),
                                stop=(k == KB - 1),
                            )
                    for c in range(TB // 512):
                        sl = slice(c * 512, (c + 1) * 512)
                        nc.scalar.activation(
                            sil[:, sl], gc[c][:], mybir.ActivationFunctionType.Silu,
                            bias=bias0[:],
                        )

                    # u runs c-outer so each 512-col half of the product is
                    # ready as soon as its 8 k-accumulation matmuls retire
                    ht = hpool.tile([128, TB], bf16, tag=f"h{f}")
                    for c in range(TB // 512):
                        sl = slice(c * 512, (c + 1) * 512)
                        u = psum.tile([128, 512], f32, tag=f"b{b + 2 + c}")
                        for k in range(KB):
                            nc.tensor.matmul(
                                u[:],
                                wt[:, 1, f2, k * 128 : (k + 1) * 128],
                                xb[:, k, sl],
                                start=(k == 0),
                                stop=(k == KB - 1),
                            )
                        nc.vector.tensor_mul(ht[:, sl], sil[:, sl], u[:])
                    hts.append(ht)

            if t + 1 < NT:
                # prefetch next block's x now: the trigger lands on the sync
                # ring AHEAD of this block's output-DMA triggers (whose deps
                # only resolve late in stage B), so the 2MB transfer runs at
                # the start of stage B instead of just-in-time at its end
                nxb = xpool.tile([128, KB, TB], bf16, tag="xb")
                nc.sync.dma_start(nxb[:], xQ[t + 1])
                xbs[t + 1] = nxb

            # ---- stage B: out[tokens, h] += hT^T @ Wd, tokens on partitions
            # m-outer/f-inner: each single-bank accumulator finishes its full
            # f-contraction and is evicted while the next accumulates, so
            # psum banks free progressively (next stage A / kernel tail never
            # wait on a bunched drain)
            for h2 in range(H // 512):
                for m in range(8):
                    acc = psum.tile(
                        [128, 512], f32, tag=f"b{m}", name=f"acc{t}_{h2}_{m}"
                    )
                    for f in range(FB):
                        nc.tensor.matmul(
                            acc[:],
                            hts[f][:, m * 128 : (m + 1) * 128],
                            wdp[h2][:, f * 512 : (f + 1) * 512],
                            start=(f == 0),
                            stop=(f == FB - 1),
                        )
                    ob = opool.tile([128, 512], bf16, tag="ob")
                    row = t * TB + m * 128
                    dst = out[row : row + 128, h2 * 512 : (h2 + 1) * 512]
                    # m<4 on ACT+scalar ring, m>=4 on DVE+sync ring: the very
                    # last eviction (m=7) takes the cheaper DVE+sync path
                    if m < 4:
                        nc.scalar.activation(
                            ob[:], acc[:], mybir.ActivationFunctionType.Copy
                        )
                        nc.scalar.dma_start(dst, ob[:])
                    else:
                        nc.vector.tensor_copy(ob[:], acc[:])
                        nc.sync.dma_start(dst, ob[:])

    nc.compile()
    return nc


def _get_module():
    if "nc" not in _CACHE:
        _CACHE["nc"] = _build_module()
    return _CACHE["nc"]


def _prep_inputs(hidden_states, Wg, Wu, Wd):
    bf16 = ml_dtypes.bfloat16
    x = np.asarray(hidden_states, dtype=np.float32).reshape(T, H)
    # xQ[t, p, k, tt] = x[t*TB+tt, 128k+p]
    xQ = np.ascontiguousarray(
        x.reshape(NT, TB, KB, 128).transpose(0, 3, 2, 1)
    ).astype(bf16)
    in_maps = []
    for e in range(N_CORES):
        # w[f, p, (k m)] = W[e, 128k+p, 128f+m], f-major tiles of FQ slices
        def _wslices(W):
            return (
                np.asarray(W, dtype=np.float32)
                .reshape(KB, 128, FB, 128)
                .transpose(2, 1, 0, 3)
                .reshape(FB // FQ, FQ, 128, KB * 128)
                .transpose(0, 2, 1, 3)  # [8, 128, FQ, 1024]
            )
        wQ = np.ascontiguousarray(
            np.stack([_wslices(Wg[e]), _wslices(Wu[e])], axis=2)
        ).astype(bf16)  # [8, 128, 2, FQ, 1024]
        # wdQ[h2, p, f*512+h] = Wd[e, 128f+p, 512*h2+h]
        wdQ = np.ascontiguousarray(
            np.asarray(Wd[e], dtype=np.float32)
            .reshape(FB, 128, 2, 512)
            .transpose(2, 1, 0, 3)
            .reshape(2, 128, FB * 512)
        ).astype(bf16)
        in_maps.append({"xQ": xQ, "wQ": wQ, "wdQ": wdQ})
    return in_maps


def _run(in_maps, trace=False, **kwargs):
    from concourse import bass_utils

    nc = _get_module()
    return bass_utils.run_bass_kernel_spmd(
        nc, in_maps, core_ids=list(range(N_CORES)), trace=trace, **kwargs
    )


def kernel(hidden_states, Wg, Wu, Wd):
    import time

    in_maps = _prep_inputs(hidden_states, Wg, Wu, Wd)
    last_exc = None
    for attempt in range(3):
        try:
            res = _run(in_maps)
            break
        except Exception as exc:  # transient device-unrecoverable wedges
            last_exc = exc
            time.sleep(5 * (attempt + 1))
    else:
        raise last_exc
    partials = np.stack(
        [np.asarray(r["out"], dtype=np.float32) for r in res.results], axis=0
    )
    total = partials.sum(axis=0, dtype=np.float32)
    return total.reshape(2, 2048, H).astype(np.float32)


# revision 22
# speedup vs baseline: 1.0054x; 1.0030x over previous
"""Dense all-expert MoE (SwiGLU) kernel for Trainium2, expert-parallel over 8 cores.

Computes: out = sum_e silu(x @ Wg[e]) * (x @ Wu[e]) @ Wd[e]
with x: [B=2, S=2048, H=1024], Wg/Wu: [8, 1024, 4096], Wd: [8, 4096, 1024].

Sharding: expert-parallel. Core e gets expert e's weights plus the full token
set; each core produces a partial [T, H] output which the host sums.

Per-core kernel (bf16 matmul inputs, fp32 PSUM accumulation):
  stage A: hT[f, :, tokens] = silu(Wg_f^T @ xT) * (Wu_f^T @ xT)   (F on partitions)
  stage B: out[tokens, h]  += hT[f]^T @ Wd_f                      (tokens on partitions)
Host pre-lays-out all operands so every DMA lands 16KB-contiguous per
partition (HWDGE is descriptor-rate bound: 2KB packets cap a ring at
~130-180 GB/s, which starves the cold start):
  xQ  [NT, 128, KB, TB]        xQ[t, p, k, tt]       = x[t*TB+tt, 128k+p]
  wQ  [8, 128, 2, FQ, 1024]    wQ[i, p, 0, f2, k*128+m] = Wg[128k+p, 128(4i+f2)+m]
                               wQ[i, p, 1, f2, ...]  = same for Wu
  wdQ [2, 128, FB*512]         wdQ[h2, p, f*512+h]   = Wd[128f+p, 512*h2+h]

Perf notes (from NTFF traces of the previous version, 1358.9us):
  - PE sem-wait bubbles (~57ns each) charge per weight-TILE switch, so wg+wu
    for FOUR f-slices ship in ONE tile: 8 waits/block instead of 32.
  - The gpsimd SWDGE ring costs a ~4.3us DRAIN at kernel end; all DMA now
    rides the sync/scalar/vector HWDGE rings only.
  - Stage B runs m-outer/f-inner so each accumulator bank finishes and is
    evicted while the next accumulates; the kernel tail after the last MM is
    one [128,512] copy + one 128KB DMA instead of a bunched 4x drain.
  - The SwiGLU mul is split into 512-col halves with the u-matmuls c-outer,
    so stage B's first LDWEIGHTS (which Tile gates on the DVE clock) is
    satisfied before stage A's last matmul retires.
  - wd preload is spread over stage-A weight tiles 2..7 of block 0 so it
    never delays the first wg/wu (and the first x chunks) at cold start.
  - Output is bf16 (host sums partials in fp32): halves the eviction DMA.
"""

import numpy as np
import ml_dtypes

T = 4096          # B*S tokens
H = 1024          # hidden
F = 4096          # ffn
E = 8             # experts
N_CORES = 8
TB = 1024         # tokens per block
NT = T // TB      # 4 token blocks
KB = H // 128     # 8 hidden slices
FB = F // 128     # 32 ffn slices
FQ = 4            # f-slices per stage-A weight tile

_CACHE = {}


def _build_module():
    from contextlib import ExitStack

    import concourse.bass as bass
    import concourse.mybir as mybir
    import concourse.tile as tile
    from concourse import bacc

    f32 = mybir.dt.float32
    bf16 = mybir.dt.bfloat16

    nc = bacc.Bacc(
        "TRN2",
        target_bir_lowering=False,
        debug=False,
        enable_asserts=False,
        num_devices=N_CORES,
    )

    xQ = nc.dram_tensor("xQ", [NT, 128, KB, TB], bf16, kind="ExternalInput").ap()
    wQ = nc.dram_tensor(
        "wQ", [FB // FQ, 128, 2, FQ, KB * 128], bf16, kind="ExternalInput"
    ).ap()
    wdQ = nc.dram_tensor("wdQ", [2, 128, FB * 512], bf16, kind="ExternalInput").ap()
    out = nc.dram_tensor("out", [T, H], bf16, kind="ExternalOutput").ap()

    # wd preload schedule: weight-tile index fq -> (wdp half, col range) of a
    # 2MB 16KB-packet DMA; all four land well before stage B reads each half
    wd_sched = {
        8: (0, 0), 12: (0, 1), 16: (1, 0), 20: (1, 1),
    }

    with tile.TileContext(nc) as tc, ExitStack() as ctx:
        xpool = ctx.enter_context(tc.tile_pool(name="xpool", bufs=1))
        wpool = ctx.enter_context(tc.tile_pool(name="wpool", bufs=2))
        dpool = ctx.enter_context(tc.tile_pool(name="dpool", bufs=1))
        hpool = ctx.enter_context(tc.tile_pool(name="hpool", bufs=1))
        spool = ctx.enter_context(tc.tile_pool(name="spool", bufs=2))
        opool = ctx.enter_context(tc.tile_pool(name="opool", bufs=4))
        cpool = ctx.enter_context(tc.tile_pool(name="cpool", bufs=1))
        # one psum pool, 8 single-bank [128,512] tags: per-bank tiles keep
        # Tile's WAR tracking at bank granularity (a start=True matmul in one
        # bank never waits an eviction copy of its neighbour).
        # stage A even f: g->(b0,b1) u->(b2,b3); odd f: b4..b7 (c halves).
        # stage B: accumulator m -> b{m}.
        psum = ctx.enter_context(tc.tile_pool(name="psum", bufs=1, space="PSUM"))

        bias0 = cpool.tile([128, 1], f32, tag="bias0")
        nc.vector.memset(bias0[:], 0.0)

        # Wd stays resident in SBUF for the whole kernel (2 x 32KB/partition)
        wdp = [
            dpool.tile([128, FB * 512], bf16, tag=f"wdp{h2}", name=f"wdp{h2}")
            for h2 in range(H // 512)
        ]

        xbs = {}
        for t in range(NT):
            # ---- stage A: hT[f] = silu(Wg_f^T xT) * (Wu_f^T xT), F on partitions
            if t == 0:
                # cold start: both HWDGE rings ramp slowly (~150KB/us), so
                # the first block's x and weights are issued as a deadline-
                # ordered interleave across BOTH rings: xb k-chunks alternate
                # scalar/sync between the weight chunks, matching the cold
                # (427ns/MM, k-outer) PE consumption order.
                xb = xpool.tile([128, KB, TB], bf16, tag="xb")
                wt0 = wpool.tile([128, 2, FQ, KB * 128], bf16, tag="w")
                # x on the scalar ring with escalating chunk sizes (bigger
                # chunks = bigger packets = faster per byte on the ramping
                # ring); weights on the sync ring in consumption order
                nc.scalar.dma_start(xb[:, 0, :], xQ[0][:, 0, :])
                nc.sync.dma_start(wt0[:, 0, 0, :], wQ[0][:, 0, 0, :])  # g f0
                nc.scalar.dma_start(xb[:, 1, :], xQ[0][:, 1, :])
                nc.sync.dma_start(wt0[:, 1, 0, :], wQ[0][:, 1, 0, :])  # u f0
                nc.scalar.dma_start(xb[:, 2:4, :], xQ[0][:, 2:4, :])
                nc.sync.dma_start(wt0[:, 0, 1, :], wQ[0][:, 0, 1, :])  # g f1
                nc.scalar.dma_start(xb[:, 4:KB, :], xQ[0][:, 4:KB, :])
                nc.sync.dma_start(wt0[:, 1, 1, :], wQ[0][:, 1, 1, :])  # u f1
                nc.sync.dma_start(wt0[:, 0, 2:FQ, :], wQ[0][:, 0, 2:FQ, :])
                nc.sync.dma_start(wt0[:, 1, 2:FQ, :], wQ[0][:, 1, 2:FQ, :])
                wts = {0: wt0}
            else:
                xb = xbs.pop(t)

            hts = []
            for fq in range(0, FB, FQ):
                # combined wg+wu tile for FQ f-slices: ONE PE sem-wait per
                # FQ slices, 16KB-contiguous per partition
                qi = fq // FQ
                if t == 0 and fq == 0:
                    wt = wts.pop(0)  # cold-start ladder issued above
                else:
                    wt = wpool.tile([128, 2, FQ, KB * 128], bf16, tag="w")
                    nc.sync.dma_start(wt[:], wQ[qi])
                if t == 0 and fq in wd_sched:
                    # wd preload spread over mid-block tiles: off the critical
                    # cold-start path, done before stage B needs each half
                    h2i, ci = wd_sched[fq]
                    sl = slice(ci * FB * 256, (ci + 1) * FB * 256)
                    nc.sync.dma_start(wdp[h2i][:, sl], wdQ[h2i][:, sl])

                for f2 in range(FQ):
                    f = fq + f2
                    b = (f % 2) * 4  # psum bank group: even f -> b0..b3, odd -> b4..b7
                    sil = spool.tile([128, TB], f32, tag="sil")
                    # g runs k-outer/c-inner: each xb k-chunk is consumed for
                    # two matmuls before the next is needed (cold start feeds
                    # at half the rate of a c-outer loop)
                    gc = [
                        psum.tile(
                            [128, 512], f32, tag=f"b{b + c}", name=f"g{t}_{f}_{c}"
                        )
                        for c in range(TB // 512)
                    ]
                    for k in range(KB):
                        for c in range(TB // 512):
                            nc.tensor.matmul(
                                gc[c][:],
                                wt[:, 0, f2, k * 128 : (k + 1) * 128],
                                xb[:, k, c * 512 : (c + 1) * 512],
                                start=(k == 0),
                                stop=(k == KB - 1),
                            )
                    for c in range(TB // 512):
                        sl = slice(c * 512, (c + 1) * 512)
                        nc.scalar.activation(
                            sil[:, sl], gc[c][:], mybir.ActivationFunctionType.Silu,
                            bias=bias0[:],
                        )

                    # u runs c-outer so each 512-col half of the product is
                    # ready as soon as its 8 k-accumulation matmuls retire
                    ht = hpool.tile([128, TB], bf16, tag=f"h{f}")
                    for c in range(TB // 512):
                        sl = slice(c * 512, (c + 1) * 512)
                        u = psum.tile([128, 512], f32, tag=f"b{b + 2 + c}")
                        for k in range(KB):
                            nc.tensor.matmul(
                                u[:],
                                wt[:, 1, f2, k * 128 : (k + 1) * 128],
                                xb[:, k, sl],
                                start=(k == 0),
                                stop=(k == KB - 1),
                            )
                        nc.vector.tensor_mul(ht[:, sl], sil[:, sl], u[:])
                    hts.append(ht)

            if t + 1 < NT:
                # prefetch next block's x now: the trigger lands on the sync
                # ring AHEAD of this block's output-DMA triggers (whose deps
                # only resolve late in stage B), so the 2MB transfer runs at
                # the start of stage B instead of just-in-time at its end
                nxb = xpool.tile([128, KB, TB], bf16, tag="xb")
                nc.sync.dma_start(nxb[:], xQ[t + 1])
                xbs[t + 1] = nxb

            # ---- stage B: out[tokens, h] += hT^T @ Wd, tokens on partitions
            # m-outer/f-inner: each single-bank accumulator finishes its full
            # f-contraction and is evicted while the next accumulates, so
            # psum banks free progressively (next stage A / kernel tail never
            # wait on a bunched drain)
            for h2 in range(H // 512):
                for m in range(8):
                    acc = psum.tile(
                        [128, 512], f32, tag=f"b{m}", name=f"acc{t}_{h2}_{m}"
                    )
                    for f in range(FB):
                        nc.tensor.matmul(
                            acc[:],
                            hts[f][:, m * 128 : (m + 1) * 128],
                            wdp[h2][:, f * 512 : (f + 1) * 512],
                            start=(f == 0),
                            stop=(f == FB - 1),
                        )
                    ob = opool.tile([128, 512], bf16, tag="ob")
                    row = t * TB + m * 128
                    dst = out[row : row + 128, h2 * 512 : (h2 + 1) * 512]
                    # m<4 on ACT+scalar ring, m>=4 on DVE+sync ring: the very
                    # last eviction (m=7) takes the cheaper DVE+sync path
                    if m < 4:
                        nc.scalar.activation(
                            ob[:], acc[:], mybir.ActivationFunctionType.Copy
                        )
                        nc.scalar.dma_start(dst, ob[:])
                    else:
                        nc.vector.tensor_copy(ob[:], acc[:])
                        nc.sync.dma_start(dst, ob[:])

    nc.compile()
    return nc


def _get_module():
    if "nc" not in _CACHE:
        _CACHE["nc"] = _build_module()
    return _CACHE["nc"]


def _prep_inputs(hidden_states, Wg, Wu, Wd):
    bf16 = ml_dtypes.bfloat16
    x = np.asarray(hidden_states, dtype=np.float32).reshape(T, H)
    # xQ[t, p, k, tt] = x[t*TB+tt, 128k+p]
    xQ = np.ascontiguousarray(
        x.reshape(NT, TB, KB, 128).transpose(0, 3, 2, 1)
    ).astype(bf16)
    in_maps = []
    for e in range(N_CORES):
        # w[f, p, (k m)] = W[e, 128k+p, 128f+m], f-major tiles of FQ slices
        def _wslices(W):
            return (
                np.asarray(W, dtype=np.float32)
                .reshape(KB, 128, FB, 128)
                .transpose(2, 1, 0, 3)
                .reshape(FB // FQ, FQ, 128, KB * 128)
                .transpose(0, 2, 1, 3)  # [8, 128, FQ, 1024]
            )
        wQ = np.ascontiguousarray(
            np.stack([_wslices(Wg[e]), _wslices(Wu[e])], axis=2)
        ).astype(bf16)  # [8, 128, 2, FQ, 1024]
        # wdQ[h2, p, f*512+h] = Wd[e, 128f+p, 512*h2+h]
        wdQ = np.ascontiguousarray(
            np.asarray(Wd[e], dtype=np.float32)
            .reshape(FB, 128, 2, 512)
            .transpose(2, 1, 0, 3)
            .reshape(2, 128, FB * 512)
        ).astype(bf16)
        in_maps.append({"xQ": xQ, "wQ": wQ, "wdQ": wdQ})
    return in_maps


def _run(in_maps, trace=False, **kwargs):
    from concourse import bass_utils

    nc = _get_module()
    return bass_utils.run_bass_kernel_spmd(
        nc, in_maps, core_ids=list(range(N_CORES)), trace=trace, **kwargs
    )


def kernel(hidden_states, Wg, Wu, Wd):
    import time

    in_maps = _prep_inputs(hidden_states, Wg, Wu, Wd)
    last_exc = None
    for attempt in range(3):
        try:
            res = _run(in_maps)
            break
        except Exception as exc:  # transient device-unrecoverable wedges
            last_exc = exc
            time.sleep(5 * (attempt + 1))
    else:
        raise last_exc
    partials = np.stack(
        [np.asarray(r["out"], dtype=np.float32) for r in res.results], axis=0
    )
    total = partials.sum(axis=0, dtype=np.float32)
    return total.reshape(2, 2048, H).astype(np.float32)
